# revision 1
# baseline (speedup 1.0000x reference)
"""Causal self-attention (B=4, T=2048, C=1024, H=16) on 8 TRN2 NeuronCores.

Sharding: hybrid batch x head tensor-parallel. Core c handles batch b = c//2
and heads [8*(c%2) : 8*(c%2)+8]. Each core computes QKV for its 8 heads over
its batch, full causal attention for those heads, and a *partial* c_proj
(contribution of its 8 heads to all 2048 tokens of its batch). The host
unshards by summing the two partial outputs of each batch pair (the c_proj
all-reduce of the pair, done at gather time); b_proj is added on-device by
the even core of each pair.

Device kernel layout choices (per core):
  - x is fed transposed (xt [C, T]) so QKV can be computed directly in the
    orientations attention wants: Q_T/K_T as [head-dim, token] (lhsT = w
    chunks, rhs = xt), V as [token, head-dim] (lhsT = xt chunks, rhs = w_v).
  - scores_T[k, q] = K_T_slice.T @ Q_T (contract over d=64). Softmax runs
    without max-subtraction (|score*scale| <= ~6 for this distribution, exp
    is safe in fp32); causal masking is a memset of fully-masked column
    ranges plus one triangular-mask multiply on the diagonal 128x128 block.
  - attn@V with lhsT = [V | ones] so PSUM row 64 accumulates the softmax
    denominators for free; normalization divides y_T by that row
    (reciprocal + gpsimd partition_broadcast + one DVE multiply).
  - All matmuls run as float32r (full-rate fp32 on the PE for N>=256).
"""

import numpy as np

import concourse.bass as bass
import concourse.mybir as mybir
import concourse.tile as tile
from concourse import bacc
from concourse.bass_utils import run_bass_kernel_spmd

B, T, C = 4, 2048, 1024
H = 16          # total heads
HL = 8          # heads per core
D = 64          # head dim
P = 128
W = 512         # matmul moving-dim window
NW = T // W     # 4 q windows
KB = T // P     # 16 k blocks
NCHUNK = C // P  # 8 contraction chunks over C
PAIRS = HL // 2  # 4 head-pairs (2 heads per 128-partition tile)
F32 = mybir.dt.float32
F32R = mybir.dt.float32r
EXP = mybir.ActivationFunctionType.Exp
N_CORES = 8

_CACHE = {}
LAST_RESULTS = None


def _r(ap):
    # tiles feeding matmuls are allocated as float32r already
    return ap


def build_nc():
    if "nc" in _CACHE:
        return _CACHE["nc"]
    nc = bacc.Bacc(
        "TRN2", target_bir_lowering=False, debug=False, num_devices=N_CORES
    )

    xt = nc.dram_tensor("xt", [C, T], F32R, kind="ExternalInput")
    wqk = nc.dram_tensor("wqk", [C, C], F32R, kind="ExternalInput")
    wv = nc.dram_tensor("wv", [C, HL * D], F32R, kind="ExternalInput")
    bqk = nc.dram_tensor("bqk", [P, 2 * PAIRS], F32, kind="ExternalInput")
    bv = nc.dram_tensor("bv", [P, HL * D], F32, kind="ExternalInput")
    wp = nc.dram_tensor("wp", [HL * D, C], F32R, kind="ExternalInput")
    bpr = nc.dram_tensor("bpr", [P, C], F32, kind="ExternalInput")
    trimask = nc.dram_tensor("trimask", [P, P], F32, kind="ExternalInput")
    onesd = nc.dram_tensor("onesd", [P, P], F32R, kind="ExternalInput")
    out = nc.dram_tensor("out", [T, C], F32, kind="ExternalOutput")

    with tile.TileContext(nc) as tc, nc.allow_low_precision(
        reason="float32r tiles for full-rate fp32 PE matmuls"
    ):
        with tc.tile_pool(name="consts", bufs=1) as consts:
            tri_t = consts.tile([P, P], F32)
            nc.sync.dma_start(tri_t[:], trimask[:])
            bqk_t = consts.tile([P, 2 * PAIRS], F32)
            nc.sync.dma_start(bqk_t[:], bqk[:])
            bv_t = consts.tile([P, HL * D], F32)
            bpr_t = consts.tile([P, C], F32)
            ones_col = consts.tile([1, D], F32R)

            with (
                tc.tile_pool(name="psum", space="PSUM", bufs=3) as psum,
                tc.tile_pool(name="qk_sb", bufs=2 * PAIRS) as qk_pool,
                tc.tile_pool(name="v_sb", bufs=1) as v_pool,
            ):
                # ---- Phase A1: V = x @ w_v + b_v, laid out [tok, d] per
                # (head, kblock) as [P, 65] slices (col 64 stays 1.0 for the
                # softmax-denominator trick).
                v_sb = v_pool.tile([P, HL * KB * 65], F32R)
                # view [P, head, kb, 65]
                v_view = v_sb[:].rearrange("p (h k c) -> p h k c", h=HL, k=KB)
                bv_view = bv_t[:].rearrange("p (h d) -> p h d", h=HL)

                with (
                    tc.tile_pool(name="wav", bufs=NCHUNK) as wav_pool,
                    tc.tile_pool(name="xtv", bufs=3) as xtv_pool,
                ):
                    wav_sb = [
                        wav_pool.tile([P, HL * D], F32R, tag="wav", name=f"wav{a}")
                        for a in range(NCHUNK)
                    ]
                    nc.sync.dma_start(wav_sb[0][:], wv[0:P, :])
                    xt_r = xt[:].rearrange("(a p) t -> p a t", p=P)
                    xtv_cache = {}

                    def xtv_get(tb):
                        if tb not in xtv_cache:
                            t = xtv_pool.tile(
                                [P, NCHUNK * P], F32R, tag="xtv",
                                name=f"xtv{tb}",
                            )
                            nc.sync.dma_start(
                                t[:].rearrange("p (a t) -> p a t", a=NCHUNK),
                                xt_r[:, :, tb * P : (tb + 1) * P],
                            )
                            xtv_cache[tb] = t
                        return xtv_cache[tb]

                    xtv_get(0)  # first rhs ahead of the remaining weights
                    for a in range(1, NCHUNK):
                        nc.sync.dma_start(
                            wav_sb[a][:], wv[a * P : (a + 1) * P, :]
                        )
                    # deferred non-critical loads: biases, ones column
                    nc.sync.dma_start(bv_t[:], bv[:])
                    nc.sync.dma_start(ones_col[:], onesd[0:1, 0:D])
                    nc.sync.dma_start(
                        v_sb[:].rearrange("p (t c) -> p t c", c=65)[:, :, 64:65],
                        onesd[:].rearrange("p (t c) -> p t c", c=1),
                    )
                    nc.sync.dma_start(bpr_t[:], bpr[:])
                    for tb in range(KB):
                        xtv = xtv_get(tb)
                        v_ps = psum.tile([P, W], F32, tag="mm")
                        for a in range(NCHUNK):
                            nc.tensor.matmul(
                                v_ps[:],
                                _r(xtv[:, a * P : (a + 1) * P]),
                                _r(wav_sb[a][:]),
                                start=(a == 0),
                                stop=(a == NCHUNK - 1),
                            )
                        nc.vector.tensor_add(
                            v_view[:, :, tb, 0:D],
                            v_ps[:].rearrange("p (h d) -> p h d", h=HL),
                            bv_view[:, :, :],
                        )

                # ---- Phase A2: Q_T / K_T = (x @ w_qk + b_qk)^T, laid out
                # [qk-col, tok]; 8 tiles of [128, T], one per head-pair
                # (blocks 0..3 = Q pairs, 4..7 = K pairs).
                qk_sb = []
                for j in range(2 * PAIRS):
                    qk_sb.append(qk_pool.tile([P, T], F32R, tag="qk", name=f"qk{j}"))
                with (
                    tc.tile_pool(name="waqk", bufs=NCHUNK) as waqk_pool,
                    tc.tile_pool(name="xtq", bufs=2) as xtq_pool,
                ):
                    waqk_sb = [
                        waqk_pool.tile([P, C], F32R, tag="waqk", name=f"waqk{a}")
                        for a in range(NCHUNK)
                    ]
                    nc.sync.dma_start(waqk_sb[0][:], wqk[0:P, :])
                    xtq_cache = {}

                    def xtq_get(w):
                        if w not in xtq_cache:
                            t = xtq_pool.tile(
                                [P, NCHUNK * W], F32R, tag="xtq", name=f"xtq{w}"
                            )
                            nc.sync.dma_start(
                                t[:].rearrange("p (a t) -> p a t", a=NCHUNK),
                                xt_r[:, :, w * W : (w + 1) * W],
                            )
                            xtq_cache[w] = t
                        return xtq_cache[w]

                    xtq_get(0)  # first rhs ahead of the remaining weights
                    for a in range(1, NCHUNK):
                        nc.sync.dma_start(
                            waqk_sb[a][:], wqk[a * P : (a + 1) * P, :]
                        )
                    for w in range(NW):
                        xtq = xtq_get(w)
                        for j in range(2 * PAIRS):
                            qk_ps = psum.tile([P, W], F32, tag="mm")
                            for a in range(NCHUNK):
                                nc.tensor.matmul(
                                    qk_ps[:],
                                    _r(waqk_sb[a][:, j * P : (j + 1) * P]),
                                    _r(xtq[:, a * W : (a + 1) * W]),
                                    start=(a == 0),
                                    stop=(a == NCHUNK - 1),
                                )
                            nc.vector.tensor_scalar(
                                out=qk_sb[j][:, w * W : (w + 1) * W],
                                in0=qk_ps[:],
                                scalar1=bqk_t[:, j : j + 1],
                                scalar2=None,
                                op0=mybir.AluOpType.add,
                            )

                # ---- Phase B: causal attention per local head.
                with (
                    tc.tile_pool(name="yt_sb", bufs=PAIRS) as yt_pool,
                    tc.tile_pool(name="attn", bufs=3) as attn_pool,
                    tc.tile_pool(name="norm", bufs=1) as norm_pool,
                ):
                    yt_sb = [yt_pool.tile([P, T], F32R, tag="yt", name=f"yt{i}") for i in range(PAIRS)]
                    for h in range(HL):
                        pr, sub = h // 2, h % 2
                        QT = qk_sb[pr]
                        KT = qk_sb[PAIRS + pr]
                        y_ps = [
                            psum.tile([65, W], F32, tag="y", bufs=4, name=f"y{h}_{i}")
                            for i in range(NW)
                        ]
                        recips = [
                            norm_pool.tile([1, W], F32R, tag="recip", bufs=4, name=f"rc{h}_{i}")
                            for i in range(NW)
                        ]
                        for kb in range(KB):
                            w0 = kb // NW
                            coff = (kb % NW) * P
                            attn_t = attn_pool.tile([P, T], F32R, tag="attn")
                            for w in range(w0, NW):
                                cs = coff if w == w0 else 0
                                s_ps = psum.tile([P, W], F32, tag="mm")
                                nc.tensor.matmul(
                                    s_ps[:, cs:W],
                                    _r(KT[sub * D : sub * D + D, kb * P : (kb + 1) * P]),
                                    _r(QT[sub * D : sub * D + D, w * W + cs : (w + 1) * W]),
                                    start=True,
                                    stop=True,
                                )
                                nc.scalar.activation(
                                    attn_t[:, w * W + cs : (w + 1) * W],
                                    s_ps[:, cs:W],
                                    EXP,
                                    scale=1.0 / np.sqrt(D),
                                )
                                if w == w0:
                                    nc.vector.tensor_mul(
                                        attn_t[:, w0 * W + coff : w0 * W + coff + P],
                                        attn_t[:, w0 * W + coff : w0 * W + coff + P],
                                        tri_t[:],
                                    )
                            for w in range(w0, NW):
                                cs = coff if w == w0 else 0
                                nc.tensor.matmul(
                                    y_ps[w][:, cs:W],
                                    _r(v_sb[:, (h * KB + kb) * 65 : (h * KB + kb) * 65 + 65]),
                                    _r(attn_t[:, w * W + cs : (w + 1) * W]),
                                    start=(kb == 0),
                                    stop=(kb == 4 * w + 3),
                                )
                            if kb % 4 == 3:
                                # window kb//4 is complete: normalize + evict
                                # its PSUM bank while later k-blocks continue.
                                w = kb // 4
                                nc.vector.reciprocal(
                                    recips[w][:], y_ps[w][64:65, :]
                                )
                                bc_ps = psum.tile(
                                    [D, W], F32, tag="bc", bufs=1, name=f"bc{h}_{w}"
                                )
                                nc.tensor.matmul(
                                    bc_ps[:],
                                    _r(ones_col[:]),
                                    _r(recips[w][:]),
                                    start=True,
                                    stop=True,
                                )
                                pbc = norm_pool.tile(
                                    [D, W], F32, tag="pbc", bufs=2, name=f"pbc{h}_{w}"
                                )
                                nc.vector.tensor_copy(pbc[:], bc_ps[:])
                                nc.vector.tensor_mul(
                                    yt_sb[pr][sub * D : sub * D + D, w * W : (w + 1) * W],
                                    y_ps[w][0:D, :],
                                    pbc[:],
                                )

                    # ---- Phase C: partial c_proj: out = y_T.T @ wp (+ bpr).
                    with (
                        tc.tile_pool(name="wp_sb", bufs=PAIRS) as wp_pool,
                        tc.tile_pool(name="osb", bufs=3) as o_pool,
                    ):
                        wp_sb = []
                        for ch in range(PAIRS):
                            t = wp_pool.tile([P, C], F32R, tag="wp", name=f"wp{ch}")
                            nc.sync.dma_start(t[:], wp[ch * P : (ch + 1) * P, :])
                            wp_sb.append(t)
                        for tb in range(KB):
                            for ew in range(C // W):
                                o_ps = psum.tile([P, W], F32, tag="mm")
                                for ch in range(PAIRS):
                                    nc.tensor.matmul(
                                        o_ps[:],
                                        _r(yt_sb[ch][:, tb * P : (tb + 1) * P]),
                                        _r(wp_sb[ch][:, ew * W : (ew + 1) * W]),
                                        start=(ch == 0),
                                        stop=(ch == PAIRS - 1),
                                    )
                                o_sb = o_pool.tile([P, W], F32, tag="osb")
                                nc.vector.tensor_add(
                                    o_sb[:], o_ps[:], bpr_t[:, ew * W : (ew + 1) * W]
                                )
                                nc.sync.dma_start(
                                    out[tb * P : (tb + 1) * P, ew * W : (ew + 1) * W],
                                    o_sb[:],
                                )

    nc.compile()
    _CACHE["nc"] = nc
    return nc


def make_in_maps(x, w_attn, b_attn, w_proj, b_proj):
    """Host-side sharding: per-core input dict."""
    x = np.ascontiguousarray(np.asarray(x, dtype=np.float32))
    w_attn = np.asarray(w_attn, dtype=np.float32)
    b_attn = np.asarray(b_attn, dtype=np.float32)
    w_proj = np.asarray(w_proj, dtype=np.float32)
    b_proj = np.asarray(b_proj, dtype=np.float32)

    trimask = np.triu(np.ones((P, P), dtype=np.float32))  # [k, q]: 1 if q >= k
    in_maps = []
    for c in range(N_CORES):
        b = c // 2
        g = c % 2
        h0 = g * HL
        # Q/K columns arranged pair-wise: [q(h0) q(h0+1) | q(h0+2) ... | k(...)]
        qcols = np.arange(h0 * D, (h0 + HL) * D)
        kcols = C + qcols
        wqk = np.concatenate(
            [w_attn[:, qcols], w_attn[:, kcols]], axis=1
        )  # [C, 1024]
        bqk_flat = np.concatenate([b_attn[qcols], b_attn[kcols]])  # [1024]
        bqk = np.ascontiguousarray(bqk_flat.reshape(2 * PAIRS, P).T)  # [128, 8]
        vcols = 2 * C + np.arange(h0 * D, (h0 + HL) * D)
        wv = np.ascontiguousarray(w_attn[:, vcols])  # [C, 512]
        bv = np.broadcast_to(b_attn[vcols], (P, HL * D)).copy()
        wp = np.ascontiguousarray(w_proj[h0 * D : (h0 + HL) * D, :])  # [512, C]
        if g == 0:
            bpr = np.broadcast_to(b_proj, (P, C)).copy()
        else:
            bpr = np.zeros((P, C), dtype=np.float32)
        in_maps.append(
            {
                "xt": np.ascontiguousarray(x[b].T),  # [C, T]
                "wqk": wqk,
                "wv": wv,
                "bqk": bqk,
                "bv": bv,
                "wp": wp,
                "bpr": bpr,
                "trimask": trimask,
                "onesd": np.ones((P, P), dtype=np.float32),
            }
        )
    return in_maps


def kernel(x, w_attn, b_attn, w_proj, b_proj, _trace=False):
    global LAST_RESULTS
    nc = build_nc()
    in_maps = make_in_maps(x, w_attn, b_attn, w_proj, b_proj)
    res = run_bass_kernel_spmd(
        nc, in_maps, list(range(N_CORES)), trace=_trace
    )
    LAST_RESULTS = res
    outs = [res.results[c]["out"] for c in range(N_CORES)]
    y = np.stack([outs[2 * b] + outs[2 * b + 1] for b in range(B)], axis=0)
    return y.astype(np.float32)



# revision 11
# speedup vs baseline: 1.1260x; 1.1260x over previous
"""Causal self-attention (B=4, T=2048, C=1024, H=16) on 8 TRN2 NeuronCores.

Sharding: hybrid batch x head tensor-parallel. Core c handles batch b = c//2
and heads [8*(c%2) : 8*(c%2)+8]. Each core computes QKV for its 8 heads over
its batch, full causal attention for those heads, and a *partial* c_proj
(contribution of its 8 heads to all 2048 tokens of its batch). The host
unshards by summing the two partial outputs of each batch pair; b_proj is
added on-device by the even core of each pair.

Schedule (flash-style, window-outer software pipeline): the 2048 query
tokens are processed in 4 windows of 512. Per window w:
  QKV(w) was computed during window w-1 (QKV(0) at startup);
  attention(w) per head walks k-blocks 0..4w+3 with scores -> exp -> attn@V;
  c_proj(w) is emitted as PE filler during window w+1.
The attention stream is Activation-engine bound (exp), so c_proj(w-1) and
QKV(w+1) matmul groups are interleaved between heads to keep the PE busy
and its p-state ramp warm.

Dtypes: x/weights/Q/K/V/attn/yt in bf16 (1 cycle/row on the PE at any
moving-dim width), psum f32, out f32. Softmax runs without max-subtraction
(|score/8| <= ~6, exp safe in fp32); denominators accumulate in psum row 64
via a ones-column in V; normalization = DVE reciprocal + Pool-engine
partition_broadcast + Pool multiply.
"""

import numpy as np
import ml_dtypes

import concourse.bass as bass
import concourse.mybir as mybir
import concourse.tile as tile
from concourse import bacc
from concourse.bass_utils import run_bass_kernel_spmd

B, T, C = 4, 2048, 1024
H = 16          # total heads
HL = 8          # heads per core
D = 64          # head dim
P = 128
W = 512         # q-window / matmul moving-dim
NW = T // W     # 4 windows
KB = T // P     # 16 k blocks
NCHUNK = C // P  # 8 contraction chunks over C
PAIRS = HL // 2  # 4 head-pairs (2 heads per 128-partition tile)
F32 = mybir.dt.float32
F32R = mybir.dt.float32r
BF16 = mybir.dt.bfloat16
EXP = mybir.ActivationFunctionType.Exp
ADD = mybir.AluOpType.add
N_CORES = 8

# fall back to PE-matmul broadcast if gpsimd partition_broadcast misbehaves
USE_POOL_BCAST = False

_CACHE = {}
LAST_RESULTS = None


def build_nc():
    if "nc" in _CACHE:
        return _CACHE["nc"]
    nc = bacc.Bacc(
        "TRN2", target_bir_lowering=False, debug=False, num_devices=N_CORES
    )

    xt = nc.dram_tensor("xt", [C, T], BF16, kind="ExternalInput")
    wqk = nc.dram_tensor("wqk", [C, C], BF16, kind="ExternalInput")
    wv = nc.dram_tensor("wv", [C, HL * D], BF16, kind="ExternalInput")
    bqk = nc.dram_tensor("bqk", [P, 2 * PAIRS], F32, kind="ExternalInput")
    bv = nc.dram_tensor("bv", [P, HL * D], F32, kind="ExternalInput")
    wp = nc.dram_tensor("wp", [HL * D, C], BF16, kind="ExternalInput")
    bpr = nc.dram_tensor("bpr", [P, C], F32, kind="ExternalInput")
    trimask = nc.dram_tensor("trimask", [P, P], BF16, kind="ExternalInput")
    onesd = nc.dram_tensor("onesd", [P, P], BF16, kind="ExternalInput")
    onesf = nc.dram_tensor("onesf", [1, D], F32R, kind="ExternalInput")
    out = nc.dram_tensor("out", [T, C], F32, kind="ExternalOutput")

    with tile.TileContext(nc) as tc, nc.allow_low_precision(
        reason="bf16 activations/weights; fp32 psum accumulation"
    ):
        with tc.tile_pool(name="consts", bufs=1) as consts:
            tri_t = consts.tile([P, P], BF16)
            bqk_t = consts.tile([P, 2 * PAIRS], F32)
            bv_t = consts.tile([P, HL * D], F32)
            bpr_t = consts.tile([P, C], F32)
            ones_col = consts.tile([1, D], F32R)

            with (
                tc.tile_pool(name="psum", space="PSUM", bufs=2) as psum,
                tc.tile_pool(name="wsb", bufs=1) as wpool,
                tc.tile_pool(name="qk_sb", bufs=1) as qk_pool,
                tc.tile_pool(name="v_sb", bufs=1) as v_pool,
                tc.tile_pool(name="xtw", bufs=2) as xtw_pool,
                tc.tile_pool(name="yt_sb", bufs=1) as yt_pool,
                tc.tile_pool(name="attn", bufs=3) as attn_pool,
                tc.tile_pool(name="norm", bufs=1) as norm_pool,
                tc.tile_pool(name="osb", bufs=3) as o_pool,
            ):
                # ---------------- persistent SBUF tensors ----------------
                wqk_sb = [
                    wpool.tile([P, C], BF16, tag=f"wqk{a}", name=f"wqk{a}")
                    for a in range(NCHUNK)
                ]
                wav_sb = [
                    wpool.tile([P, HL * D], BF16, tag=f"wav{a}", name=f"wav{a}")
                    for a in range(NCHUNK)
                ]
                wp_sb = [
                    wpool.tile([P, C], BF16, tag=f"wp{c}", name=f"wp{c}")
                    for c in range(PAIRS)
                ]
                # K: full history per pair-tile; Q: per-window (2 window bufs)
                kt_sb = [
                    qk_pool.tile([P, T], BF16, tag=f"kt{j}", name=f"kt{j}")
                    for j in range(PAIRS)
                ]
                qt_sb = [
                    [
                        qk_pool.tile(
                            [P, W], BF16, tag=f"qt{j}", bufs=2, name=f"qt{j}_{wi}"
                        )
                        for j in range(PAIRS)
                    ]
                    for wi in range(NW)
                ]
                # V per (head, kblock): [tok, 64 + ones column]
                v_sb = v_pool.tile([P, HL * KB * 65], BF16)
                v_view = v_sb[:].rearrange("p (h k c) -> p h k c", h=HL, k=KB)
                bv_view = bv_t[:].rearrange("p (h d) -> p h d", h=HL)
                yt_sb = [
                    yt_pool.tile([P, T], BF16, tag=f"yt{c}", name=f"yt{c}")
                    for c in range(PAIRS)
                ]
                xtw_tiles = [
                    xtw_pool.tile(
                        [P, NCHUNK * W], BF16, tag="xtw", name=f"xtw{wi}"
                    )
                    for wi in range(NW)
                ]
                xt_r = xt[:].rearrange("(a p) t -> p a t", p=P)

                # ---------------- DMA issue helpers ----------------
                def load_xtw(wi):
                    nc.sync.dma_start(
                        xtw_tiles[wi][:].rearrange("p (a t) -> p a t", a=NCHUNK),
                        xt_r[:, :, wi * W : (wi + 1) * W],
                    )

                # startup DMAs: first wqk chunk + first x window ASAP
                nc.sync.dma_start(wqk_sb[0][:], wqk[0:P, :])
                load_xtw(0)
                for a in range(1, NCHUNK):
                    nc.sync.dma_start(wqk_sb[a][:], wqk[a * P : (a + 1) * P, :])
                nc.sync.dma_start(bqk_t[:], bqk[:])
                for a in range(NCHUNK):
                    nc.sync.dma_start(wav_sb[a][:], wv[a * P : (a + 1) * P, :])
                nc.sync.dma_start(bv_t[:], bv[:])
                nc.sync.dma_start(tri_t[:], trimask[:])
                nc.sync.dma_start(ones_col[:], onesf[:])
                # ones column of V (col 64 of each [P,65] slice): strided
                # 2-byte DMA is fragile on HW; memset on the Pool engine.
                nc.gpsimd.memset(
                    v_sb[:].rearrange("p (t c) -> p t c", c=65)[:, :, 64:65],
                    1.0,
                )
                for ch in range(PAIRS):
                    nc.sync.dma_start(wp_sb[ch][:], wp[ch * P : (ch + 1) * P, :])
                nc.sync.dma_start(bpr_t[:], bpr[:])

                # ---------------- emission groups ----------------
                def qk_group(wi, j):
                    """One [128ch, 512tok] QK projection group for window wi."""
                    xtw = xtw_tiles[wi][:].rearrange(
                        "p (a t) -> p a t", a=NCHUNK
                    )
                    ps = psum.tile([P, W], F32, tag="mm", name=f"qk{wi}_{j}")
                    for a in range(NCHUNK):
                        nc.tensor.matmul(
                            ps[:],
                            wqk_sb[a][:, j * P : (j + 1) * P],
                            xtw[:, a, :],
                            start=(a == 0),
                            stop=(a == NCHUNK - 1),
                        )
                    if j < PAIRS:
                        dst = qt_sb[wi][j][:, :]
                    else:
                        dst = kt_sb[j - PAIRS][:, wi * W : (wi + 1) * W]
                    nc.vector.tensor_scalar(
                        out=dst,
                        in0=ps[:],
                        scalar1=bqk_t[:, j : j + 1],
                        scalar2=None,
                        op0=ADD,
                    )

                def v_group(wi, tb):
                    """One [128tok, 512hd] V projection group (k-block 4wi+tb)."""
                    xtw = xtw_tiles[wi][:].rearrange(
                        "p (a t) -> p a t", a=NCHUNK
                    )
                    ps = psum.tile([P, W], F32, tag="mm", name=f"v{wi}_{tb}")
                    for a in range(NCHUNK):
                        nc.tensor.matmul(
                            ps[:],
                            xtw[:, a, tb * P : (tb + 1) * P],
                            wav_sb[a][:],
                            start=(a == 0),
                            stop=(a == NCHUNK - 1),
                        )
                    kb = 4 * wi + tb
                    nc.vector.tensor_add(
                        v_view[:, :, kb, 0:D],
                        ps[:].rearrange("p (h d) -> p h d", h=HL),
                        bv_view[:, :, :],
                    )

                def cproj_group(wi, tb, ew):
                    """c_proj for token block 4wi+tb, output cols [512ew, 512ew+512)."""
                    gb = 4 * wi + tb
                    ps = psum.tile([P, W], F32, tag="mm", name=f"o{gb}_{ew}")
                    for ch in range(PAIRS):
                        nc.tensor.matmul(
                            ps[:],
                            yt_sb[ch][:, gb * P : (gb + 1) * P],
                            wp_sb[ch][:, ew * W : (ew + 1) * W],
                            start=(ch == 0),
                            stop=(ch == PAIRS - 1),
                        )
                    o_sb = o_pool.tile([P, W], F32, tag="osb")
                    nc.vector.tensor_add(
                        o_sb[:], ps[:], bpr_t[:, ew * W : (ew + 1) * W]
                    )
                    nc.sync.dma_start(
                        out[gb * P : (gb + 1) * P, ew * W : (ew + 1) * W],
                        o_sb[:],
                    )

                def attention_head(h, wi):
                    """Causal attention for head h over q-window wi."""
                    pr, sub = h // 2, h % 2
                    QT = qt_sb[wi][pr]
                    KT = kt_sb[pr]
                    nkb = 4 * wi + 4
                    y_ps = psum.tile(
                        [65, W], F32, tag="y", bufs=2, name=f"y{h}_{wi}"
                    )
                    # k-block pairs: 2wi non-diag pairs + 2 diagonal pairs
                    for pi in range(nkb // 2):
                        kb0 = 2 * pi
                        s_ps = psum.tile(
                            [P, 2 * W], F32, tag="s", bufs=2,
                            name=f"s{h}_{wi}_{pi}",
                        )
                        attn_t = attn_pool.tile([P, 2 * W], BF16, tag="attn")
                        diag = kb0 >= 4 * wi
                        for sl in range(2):
                            kb = kb0 + sl
                            cs = (kb - 4 * wi) * P if kb >= 4 * wi else 0
                            nc.tensor.matmul(
                                s_ps[:, sl * W + cs : (sl + 1) * W],
                                KT[sub * D : sub * D + D, kb * P : (kb + 1) * P],
                                QT[sub * D : sub * D + D, cs:W],
                                start=True,
                                stop=True,
                            )
                        if not diag:
                            # one 1024-wide exp over both k-blocks
                            nc.scalar.activation(
                                attn_t[:, :],
                                s_ps[:, :],
                                EXP,
                                scale=1.0 / np.sqrt(D),
                            )
                        else:
                            for sl in range(2):
                                kb = kb0 + sl
                                cs = (kb - 4 * wi) * P
                                nc.scalar.activation(
                                    attn_t[:, sl * W + cs : (sl + 1) * W],
                                    s_ps[:, sl * W + cs : (sl + 1) * W],
                                    EXP,
                                    scale=1.0 / np.sqrt(D),
                                )
                                nc.vector.tensor_mul(
                                    attn_t[:, sl * W + cs : sl * W + cs + P],
                                    attn_t[:, sl * W + cs : sl * W + cs + P],
                                    tri_t[:],
                                )
                        for sl in range(2):
                            kb = kb0 + sl
                            cs = (kb - 4 * wi) * P if kb >= 4 * wi else 0
                            nc.tensor.matmul(
                                y_ps[:, cs:W],
                                v_sb[:, (h * KB + kb) * 65 : (h * KB + kb) * 65 + 65],
                                attn_t[:, sl * W + cs : (sl + 1) * W],
                                start=(kb == 0),
                                stop=(kb == nkb - 1),
                            )
                    # normalize: yt = y / denom (psum row 64)
                    recip = norm_pool.tile(
                        [1, W], F32R, tag="recip", bufs=2, name=f"rc{h}_{wi}"
                    )
                    nc.vector.reciprocal(recip[:], y_ps[64:65, :])
                    pbc = norm_pool.tile(
                        [D, W], F32, tag="pbc", bufs=2, name=f"pbc{h}_{wi}"
                    )
                    if USE_POOL_BCAST:
                        # gpsimd cannot read PSUM: broadcast on Pool (SBUF
                        # only), multiply on DVE (reads psum y).
                        nc.gpsimd.partition_broadcast(pbc[:], recip[:])
                        nc.vector.tensor_mul(
                            yt_sb[pr][sub * D : sub * D + D, wi * W : (wi + 1) * W],
                            y_ps[0:D, :],
                            pbc[:],
                        )
                    else:
                        bc_ps = psum.tile(
                            [D, W], F32, tag="mm", name=f"bc{h}_{wi}"
                        )
                        nc.tensor.matmul(
                            bc_ps[:], ones_col[:], recip[:], start=True, stop=True
                        )
                        nc.vector.tensor_copy(pbc[:], bc_ps[:])
                        nc.vector.tensor_mul(
                            yt_sb[pr][sub * D : sub * D + D, wi * W : (wi + 1) * W],
                            y_ps[0:D, :],
                            pbc[:],
                        )

                # ---------------- main schedule ----------------
                # startup: QKV(0)
                for j in range(2 * PAIRS):
                    qk_group(0, j)
                for tb in range(4):
                    v_group(0, tb)

                for wi in range(NW):
                    if wi + 1 < NW:
                        load_xtw(wi + 1)
                    fillers = []
                    if wi > 0:
                        for tb in range(4):
                            for ew in range(C // W):
                                fillers.append(
                                    (cproj_group, (wi - 1, tb, ew))
                                )
                    if wi + 1 < NW:
                        for j in range(2 * PAIRS):
                            fillers.append((qk_group, (wi + 1, j)))
                        for tb in range(4):
                            fillers.append((v_group, (wi + 1, tb)))
                    nf = len(fillers)
                    emitted = 0
                    for h in range(HL):
                        # spread fillers evenly across head boundaries
                        want = (h + 1) * nf // HL
                        while emitted < want:
                            fn, args = fillers[emitted]
                            fn(*args)
                            emitted += 1
                        attention_head(h, wi)
                    while emitted < nf:
                        fn, args = fillers[emitted]
                        fn(*args)
                        emitted += 1

                # tail: c_proj of the last window
                for tb in range(4):
                    for ew in range(C // W):
                        cproj_group(NW - 1, tb, ew)

    nc.compile()
    _CACHE["nc"] = nc
    return nc


def make_in_maps(x, w_attn, b_attn, w_proj, b_proj):
    """Host-side sharding: per-core input dict."""
    x = np.ascontiguousarray(np.asarray(x, dtype=np.float32))
    w_attn = np.asarray(w_attn, dtype=np.float32)
    b_attn = np.asarray(b_attn, dtype=np.float32)
    w_proj = np.asarray(w_proj, dtype=np.float32)
    b_proj = np.asarray(b_proj, dtype=np.float32)
    bf = ml_dtypes.bfloat16

    trimask = np.triu(np.ones((P, P), dtype=np.float32))  # [k, q]: 1 if q >= k
    in_maps = []
    for c in range(N_CORES):
        b = c // 2
        g = c % 2
        h0 = g * HL
        # Q/K columns arranged pair-wise: [q(h0) q(h0+1) | q(h0+2) ... | k(...)]
        qcols = np.arange(h0 * D, (h0 + HL) * D)
        kcols = C + qcols
        wqk = np.concatenate(
            [w_attn[:, qcols], w_attn[:, kcols]], axis=1
        )  # [C, 1024]
        bqk_flat = np.concatenate([b_attn[qcols], b_attn[kcols]])  # [1024]
        bqk = np.ascontiguousarray(bqk_flat.reshape(2 * PAIRS, P).T)  # [128, 8]
        vcols = 2 * C + np.arange(h0 * D, (h0 + HL) * D)
        wv = np.ascontiguousarray(w_attn[:, vcols])  # [C, 512]
        bv = np.broadcast_to(b_attn[vcols], (P, HL * D)).copy()
        wp = np.ascontiguousarray(w_proj[h0 * D : (h0 + HL) * D, :])  # [512, C]
        if g == 0:
            bpr = np.broadcast_to(b_proj, (P, C)).copy()
        else:
            bpr = np.zeros((P, C), dtype=np.float32)
        in_maps.append(
            {
                "xt": np.ascontiguousarray(x[b].T).astype(bf),  # [C, T]
                "wqk": wqk.astype(bf),
                "wv": wv.astype(bf),
                "bqk": bqk,
                "bv": bv,
                "wp": wp.astype(bf),
                "bpr": bpr,
                "trimask": trimask.astype(bf),
                "onesd": np.ones((P, P), dtype=bf),
                "onesf": np.ones((1, D), dtype=np.float32),
            }
        )
    return in_maps


def kernel(x, w_attn, b_attn, w_proj, b_proj, _trace=False):
    global LAST_RESULTS
    nc = build_nc()
    in_maps = make_in_maps(x, w_attn, b_attn, w_proj, b_proj)
    res = run_bass_kernel_spmd(
        nc, in_maps, list(range(N_CORES)), trace=_trace
    )
    LAST_RESULTS = res
    outs = [res.results[c]["out"] for c in range(N_CORES)]
    y = np.stack([outs[2 * b] + outs[2 * b + 1] for b in range(B)], axis=0)
    return y.astype(np.float32)


# revision 13
# speedup vs baseline: 1.3162x; 1.1689x over previous
"""Causal self-attention (B=4, T=2048, C=1024, H=16) on 8 TRN2 NeuronCores.

Sharding: hybrid batch x head tensor-parallel. Core c handles batch b = c//2
and heads [8*(c%2) : 8*(c%2)+8]. Each core computes QKV for its 8 heads over
its batch, full causal attention for those heads, and a *partial* c_proj
(contribution of its 8 heads to all 2048 tokens of its batch). The host
unshards by summing the two partial outputs of each batch pair; b_proj is
added on-device by the even core of each pair.

Schedule (flash-style, window-outer software pipeline): the 2048 query
tokens are processed in 4 windows of 512. Per window w:
  QKV(w) was computed during window w-1 (QKV(0) at startup);
  attention(w) per head walks k-blocks 0..4w+3 with scores -> exp -> attn@V;
  c_proj(w) is emitted as PE filler during window w+1.
The attention stream is Activation-engine bound (exp), so c_proj(w-1) and
QKV(w+1) matmul groups are interleaved between heads to keep the PE busy
and its p-state ramp warm.

Dtypes: x/weights/Q/K/V/attn/yt in bf16 (1 cycle/row on the PE at any
moving-dim width), psum f32, out f32. Softmax runs without max-subtraction
(|score/8| <= ~6, exp safe in fp32); denominators accumulate in psum row 64
via a ones-column in V; normalization = DVE reciprocal + Pool-engine
partition_broadcast + Pool multiply.
"""

import numpy as np
import ml_dtypes

import concourse.bass as bass
import concourse.mybir as mybir
import concourse.tile as tile
from concourse import bacc
from concourse.bass_utils import run_bass_kernel_spmd

B, T, C = 4, 2048, 1024
H = 16          # total heads
HL = 8          # heads per core
D = 64          # head dim
P = 128
W = 512         # q-window / matmul moving-dim
NW = T // W     # 4 windows
KB = T // P     # 16 k blocks
NCHUNK = C // P  # 8 contraction chunks over C
PAIRS = HL // 2  # 4 head-pairs (2 heads per 128-partition tile)
F32 = mybir.dt.float32
F32R = mybir.dt.float32r
BF16 = mybir.dt.bfloat16
EXP = mybir.ActivationFunctionType.Exp
ADD = mybir.AluOpType.add
N_CORES = 8

# fall back to PE-matmul broadcast if gpsimd partition_broadcast misbehaves
USE_POOL_BCAST = True

_CACHE = {}
LAST_RESULTS = None


def build_nc():
    if "nc" in _CACHE:
        return _CACHE["nc"]
    nc = bacc.Bacc(
        "TRN2", target_bir_lowering=False, debug=False, num_devices=N_CORES
    )

    xt = nc.dram_tensor("xt", [C, T], BF16, kind="ExternalInput")
    wqk = nc.dram_tensor("wqk", [C, C], BF16, kind="ExternalInput")
    wv = nc.dram_tensor("wv", [C, HL * D], BF16, kind="ExternalInput")
    bqk = nc.dram_tensor("bqk", [P, 2 * PAIRS], F32, kind="ExternalInput")
    bv = nc.dram_tensor("bv", [P, HL * D], F32, kind="ExternalInput")
    wp = nc.dram_tensor("wp", [HL * D, C], BF16, kind="ExternalInput")
    bpr = nc.dram_tensor("bpr", [P, C], F32, kind="ExternalInput")
    trimask = nc.dram_tensor("trimask", [P, P], BF16, kind="ExternalInput")
    onesd = nc.dram_tensor("onesd", [P, P], BF16, kind="ExternalInput")
    onesf = nc.dram_tensor("onesf", [1, D], F32R, kind="ExternalInput")
    out = nc.dram_tensor("out", [T, C], F32, kind="ExternalOutput")

    with tile.TileContext(nc) as tc, nc.allow_low_precision(
        reason="bf16 activations/weights; fp32 psum accumulation"
    ):
        with tc.tile_pool(name="consts", bufs=1) as consts:
            tri_t = consts.tile([P, P], BF16)
            bqk_t = consts.tile([P, 2 * PAIRS], F32)
            bv_t = consts.tile([P, HL * D], F32)
            bpr_t = consts.tile([P, C], F32)
            ones_col = consts.tile([1, D], F32R)

            with (
                tc.tile_pool(name="psum", space="PSUM", bufs=2) as psum,
                tc.tile_pool(name="wsb", bufs=1) as wpool,
                tc.tile_pool(name="qk_sb", bufs=1) as qk_pool,
                tc.tile_pool(name="v_sb", bufs=1) as v_pool,
                tc.tile_pool(name="xtw", bufs=2) as xtw_pool,
                tc.tile_pool(name="yt_sb", bufs=1) as yt_pool,
                tc.tile_pool(name="attn", bufs=3) as attn_pool,
                tc.tile_pool(name="norm", bufs=1) as norm_pool,
                tc.tile_pool(name="osb", bufs=3) as o_pool,
            ):
                # ---------------- persistent SBUF tensors ----------------
                wqk_sb = [
                    wpool.tile([P, C], BF16, tag=f"wqk{a}", name=f"wqk{a}")
                    for a in range(NCHUNK)
                ]
                wav_sb = [
                    wpool.tile([P, HL * D], BF16, tag=f"wav{a}", name=f"wav{a}")
                    for a in range(NCHUNK)
                ]
                wp_sb = [
                    wpool.tile([P, C], BF16, tag=f"wp{c}", name=f"wp{c}")
                    for c in range(PAIRS)
                ]
                # K: full history per pair-tile; Q: per-window (2 window bufs)
                kt_sb = [
                    qk_pool.tile([P, T], BF16, tag=f"kt{j}", name=f"kt{j}")
                    for j in range(PAIRS)
                ]
                qt_sb = [
                    [
                        qk_pool.tile(
                            [P, W], BF16, tag=f"qt{j}", bufs=2, name=f"qt{j}_{wi}"
                        )
                        for j in range(PAIRS)
                    ]
                    for wi in range(NW)
                ]
                # V per (head, kblock): [tok, 64 + ones column]
                v_sb = v_pool.tile([P, HL * KB * 65], BF16)
                v_view = v_sb[:].rearrange("p (h k c) -> p h k c", h=HL, k=KB)
                bv_view = bv_t[:].rearrange("p (h d) -> p h d", h=HL)
                yt_sb = [
                    yt_pool.tile([P, T], BF16, tag=f"yt{c}", name=f"yt{c}")
                    for c in range(PAIRS)
                ]
                xtw_tiles = [
                    xtw_pool.tile(
                        [P, NCHUNK * W], BF16, tag="xtw", name=f"xtw{wi}"
                    )
                    for wi in range(NW)
                ]
                xt_r = xt[:].rearrange("(a p) t -> p a t", p=P)

                # ---------------- DMA issue helpers ----------------
                def load_xtw(wi):
                    nc.sync.dma_start(
                        xtw_tiles[wi][:].rearrange("p (a t) -> p a t", a=NCHUNK),
                        xt_r[:, :, wi * W : (wi + 1) * W],
                    )

                # startup DMAs: first wqk chunk + first x window ASAP
                nc.sync.dma_start(wqk_sb[0][:], wqk[0:P, :])
                load_xtw(0)
                for a in range(1, NCHUNK):
                    nc.sync.dma_start(wqk_sb[a][:], wqk[a * P : (a + 1) * P, :])
                nc.sync.dma_start(bqk_t[:], bqk[:])
                for a in range(NCHUNK):
                    nc.sync.dma_start(wav_sb[a][:], wv[a * P : (a + 1) * P, :])
                nc.sync.dma_start(bv_t[:], bv[:])
                nc.sync.dma_start(tri_t[:], trimask[:])
                nc.sync.dma_start(ones_col[:], onesf[:])
                # ones column of V (col 64 of each [P,65] slice): strided
                # 2-byte DMA is fragile on HW; memset on the Pool engine.
                nc.gpsimd.memset(
                    v_sb[:].rearrange("p (t c) -> p t c", c=65)[:, :, 64:65],
                    1.0,
                )
                for ch in range(PAIRS):
                    nc.sync.dma_start(wp_sb[ch][:], wp[ch * P : (ch + 1) * P, :])
                nc.sync.dma_start(bpr_t[:], bpr[:])

                # ---------------- emission groups ----------------
                def qk_group(wi, j):
                    """One [128ch, 512tok] QK projection group for window wi."""
                    xtw = xtw_tiles[wi][:].rearrange(
                        "p (a t) -> p a t", a=NCHUNK
                    )
                    ps = psum.tile([P, W], F32, tag="mm", name=f"qk{wi}_{j}")
                    for a in range(NCHUNK):
                        nc.tensor.matmul(
                            ps[:],
                            wqk_sb[a][:, j * P : (j + 1) * P],
                            xtw[:, a, :],
                            start=(a == 0),
                            stop=(a == NCHUNK - 1),
                        )
                    if j < PAIRS:
                        dst = qt_sb[wi][j][:, :]
                    else:
                        dst = kt_sb[j - PAIRS][:, wi * W : (wi + 1) * W]
                    nc.vector.tensor_scalar(
                        out=dst,
                        in0=ps[:],
                        scalar1=bqk_t[:, j : j + 1],
                        scalar2=None,
                        op0=ADD,
                    )

                def v_group(wi, tb):
                    """One [128tok, 512hd] V projection group (k-block 4wi+tb)."""
                    xtw = xtw_tiles[wi][:].rearrange(
                        "p (a t) -> p a t", a=NCHUNK
                    )
                    ps = psum.tile([P, W], F32, tag="mm", name=f"v{wi}_{tb}")
                    for a in range(NCHUNK):
                        nc.tensor.matmul(
                            ps[:],
                            xtw[:, a, tb * P : (tb + 1) * P],
                            wav_sb[a][:],
                            start=(a == 0),
                            stop=(a == NCHUNK - 1),
                        )
                    kb = 4 * wi + tb
                    nc.vector.tensor_add(
                        v_view[:, :, kb, 0:D],
                        ps[:].rearrange("p (h d) -> p h d", h=HL),
                        bv_view[:, :, :],
                    )

                def cproj_group(wi, tb, ew):
                    """c_proj for token block 4wi+tb, output cols [512ew, 512ew+512)."""
                    gb = 4 * wi + tb
                    ps = psum.tile([P, W], F32, tag="mm", name=f"o{gb}_{ew}")
                    for ch in range(PAIRS):
                        nc.tensor.matmul(
                            ps[:],
                            yt_sb[ch][:, gb * P : (gb + 1) * P],
                            wp_sb[ch][:, ew * W : (ew + 1) * W],
                            start=(ch == 0),
                            stop=(ch == PAIRS - 1),
                        )
                    o_sb = o_pool.tile([P, W], F32, tag="osb")
                    nc.vector.tensor_add(
                        o_sb[:], ps[:], bpr_t[:, ew * W : (ew + 1) * W]
                    )
                    nc.sync.dma_start(
                        out[gb * P : (gb + 1) * P, ew * W : (ew + 1) * W],
                        o_sb[:],
                    )

                def attention_head(h, wi):
                    """Causal attention for head h over q-window wi."""
                    pr, sub = h // 2, h % 2
                    QT = qt_sb[wi][pr]
                    KT = kt_sb[pr]
                    nkb = 4 * wi + 4
                    y_ps = psum.tile(
                        [65, W], F32, tag="y", bufs=2, name=f"y{h}_{wi}"
                    )
                    # k-block pairs: 2wi non-diag pairs + 2 diagonal pairs
                    for pi in range(nkb // 2):
                        kb0 = 2 * pi
                        s_ps = psum.tile(
                            [P, 2 * W], F32, tag="s", bufs=2,
                            name=f"s{h}_{wi}_{pi}",
                        )
                        attn_t = attn_pool.tile([P, 2 * W], BF16, tag="attn")
                        diag = kb0 >= 4 * wi
                        for sl in range(2):
                            kb = kb0 + sl
                            cs = (kb - 4 * wi) * P if kb >= 4 * wi else 0
                            nc.tensor.matmul(
                                s_ps[:, sl * W + cs : (sl + 1) * W],
                                KT[sub * D : sub * D + D, kb * P : (kb + 1) * P],
                                QT[sub * D : sub * D + D, cs:W],
                                start=True,
                                stop=True,
                            )
                        if not diag:
                            # one 1024-wide exp over both k-blocks
                            nc.scalar.activation(
                                attn_t[:, :],
                                s_ps[:, :],
                                EXP,
                                scale=1.0 / np.sqrt(D),
                            )
                        else:
                            for sl in range(2):
                                kb = kb0 + sl
                                cs = (kb - 4 * wi) * P
                                nc.scalar.activation(
                                    attn_t[:, sl * W + cs : (sl + 1) * W],
                                    s_ps[:, sl * W + cs : (sl + 1) * W],
                                    EXP,
                                    scale=1.0 / np.sqrt(D),
                                )
                                nc.vector.tensor_mul(
                                    attn_t[:, sl * W + cs : sl * W + cs + P],
                                    attn_t[:, sl * W + cs : sl * W + cs + P],
                                    tri_t[:],
                                )
                        for sl in range(2):
                            kb = kb0 + sl
                            cs = (kb - 4 * wi) * P if kb >= 4 * wi else 0
                            nc.tensor.matmul(
                                y_ps[:, cs:W],
                                v_sb[:, (h * KB + kb) * 65 : (h * KB + kb) * 65 + 65],
                                attn_t[:, sl * W + cs : (sl + 1) * W],
                                start=(kb == 0),
                                stop=(kb == nkb - 1),
                            )
                    # normalize: yt = y / denom (psum row 64)
                    recip = norm_pool.tile(
                        [1, W], F32R, tag="recip", bufs=2, name=f"rc{h}_{wi}"
                    )
                    nc.vector.reciprocal(recip[:], y_ps[64:65, :])
                    pbc = norm_pool.tile(
                        [D, W], F32R, tag="pbc", bufs=2, name=f"pbc{h}_{wi}"
                    )
                    if USE_POOL_BCAST:
                        # gpsimd cannot read PSUM: broadcast on Pool (SBUF
                        # only), multiply on DVE (reads psum y).
                        nc.gpsimd.partition_broadcast(pbc[:], recip[:])
                        nc.vector.tensor_mul(
                            yt_sb[pr][sub * D : sub * D + D, wi * W : (wi + 1) * W],
                            y_ps[0:D, :],
                            pbc[:],
                        )
                    else:
                        bc_ps = psum.tile(
                            [D, W], F32, tag="mm", name=f"bc{h}_{wi}"
                        )
                        nc.tensor.matmul(
                            bc_ps[:], ones_col[:], recip[:], start=True, stop=True
                        )
                        nc.vector.tensor_copy(pbc[:], bc_ps[:])
                        nc.vector.tensor_mul(
                            yt_sb[pr][sub * D : sub * D + D, wi * W : (wi + 1) * W],
                            y_ps[0:D, :],
                            pbc[:],
                        )

                # ---------------- main schedule ----------------
                # startup: QKV(0)
                for j in range(2 * PAIRS):
                    qk_group(0, j)
                for tb in range(4):
                    v_group(0, tb)

                for wi in range(NW):
                    if wi + 1 < NW:
                        load_xtw(wi + 1)
                    fillers = []
                    if wi > 0:
                        for tb in range(4):
                            for ew in range(C // W):
                                fillers.append(
                                    (cproj_group, (wi - 1, tb, ew))
                                )
                    if wi + 1 < NW:
                        for j in range(2 * PAIRS):
                            fillers.append((qk_group, (wi + 1, j)))
                        for tb in range(4):
                            fillers.append((v_group, (wi + 1, tb)))
                    nf = len(fillers)
                    emitted = 0
                    for h in range(HL):
                        # spread fillers evenly across head boundaries
                        want = (h + 1) * nf // HL
                        while emitted < want:
                            fn, args = fillers[emitted]
                            fn(*args)
                            emitted += 1
                        attention_head(h, wi)
                    while emitted < nf:
                        fn, args = fillers[emitted]
                        fn(*args)
                        emitted += 1

                # tail: c_proj of the last window
                for tb in range(4):
                    for ew in range(C // W):
                        cproj_group(NW - 1, tb, ew)

    nc.compile()
    _CACHE["nc"] = nc
    return nc


def make_in_maps(x, w_attn, b_attn, w_proj, b_proj):
    """Host-side sharding: per-core input dict."""
    x = np.ascontiguousarray(np.asarray(x, dtype=np.float32))
    w_attn = np.asarray(w_attn, dtype=np.float32)
    b_attn = np.asarray(b_attn, dtype=np.float32)
    w_proj = np.asarray(w_proj, dtype=np.float32)
    b_proj = np.asarray(b_proj, dtype=np.float32)
    bf = ml_dtypes.bfloat16

    trimask = np.triu(np.ones((P, P), dtype=np.float32))  # [k, q]: 1 if q >= k
    in_maps = []
    for c in range(N_CORES):
        b = c // 2
        g = c % 2
        h0 = g * HL
        # Q/K columns arranged pair-wise: [q(h0) q(h0+1) | q(h0+2) ... | k(...)]
        qcols = np.arange(h0 * D, (h0 + HL) * D)
        kcols = C + qcols
        wqk = np.concatenate(
            [w_attn[:, qcols], w_attn[:, kcols]], axis=1
        )  # [C, 1024]
        bqk_flat = np.concatenate([b_attn[qcols], b_attn[kcols]])  # [1024]
        bqk = np.ascontiguousarray(bqk_flat.reshape(2 * PAIRS, P).T)  # [128, 8]
        vcols = 2 * C + np.arange(h0 * D, (h0 + HL) * D)
        wv = np.ascontiguousarray(w_attn[:, vcols])  # [C, 512]
        bv = np.broadcast_to(b_attn[vcols], (P, HL * D)).copy()
        wp = np.ascontiguousarray(w_proj[h0 * D : (h0 + HL) * D, :])  # [512, C]
        if g == 0:
            bpr = np.broadcast_to(b_proj, (P, C)).copy()
        else:
            bpr = np.zeros((P, C), dtype=np.float32)
        in_maps.append(
            {
                "xt": np.ascontiguousarray(x[b].T).astype(bf),  # [C, T]
                "wqk": wqk.astype(bf),
                "wv": wv.astype(bf),
                "bqk": bqk,
                "bv": bv,
                "wp": wp.astype(bf),
                "bpr": bpr,
                "trimask": trimask.astype(bf),
                "onesd": np.ones((P, P), dtype=bf),
                "onesf": np.ones((1, D), dtype=np.float32),
            }
        )
    return in_maps


def kernel(x, w_attn, b_attn, w_proj, b_proj, _trace=False):
    global LAST_RESULTS
    nc = build_nc()
    in_maps = make_in_maps(x, w_attn, b_attn, w_proj, b_proj)
    res = run_bass_kernel_spmd(
        nc, in_maps, list(range(N_CORES)), trace=_trace
    )
    LAST_RESULTS = res
    outs = [res.results[c]["out"] for c in range(N_CORES)]
    y = np.stack([outs[2 * b] + outs[2 * b + 1] for b in range(B)], axis=0)
    return y.astype(np.float32)
